# revision 14
# baseline (speedup 1.0000x reference)
"""MoE routing kernel (softmax -> top-8 -> sort/dispatch by expert) on 8 TRN2 cores.

Strategy: every core computes the full routing (cheap: [8192, 64]) and builds the
global expert-sorted metadata on-device; output rows are sharded by global output
position (8192 rows per core), so each core does an indirect row gather + scale +
contiguous store for its slice and the host just concatenates.
"""

import numpy as np

import concourse.bacc as bacc
import concourse.bass as bass
import concourse.mybir as mybir
import concourse.tile as tile
from concourse.bass_utils import run_bass_kernel_spmd
from concourse.masks import make_identity, make_upper_triangular

F32 = mybir.dt.float32
I32 = mybir.dt.int32
AF = mybir.ActivationFunctionType
OP = mybir.AluOpType
AX = mybir.AxisListType

N = 8192          # tokens
E = 64            # experts
K = 8             # top-k
H = 2048          # hidden
P = 128
NT = N // P       # token tiles
C = 8             # cores
NK = N * K        # total dispatched rows
OPC = NK // C     # out rows per core
OT = OPC // P     # out tiles per core
LBL_COEF = 0.01


def _build(nc, scale_split=2):
    inputs_flat = nc.dram_tensor("inputs_flat", [N, H], F32, kind="ExternalInput")
    router_logits = nc.dram_tensor("router_logits", [N, E], F32, kind="ExternalInput")
    r0_d = nc.dram_tensor("r0", [P, 1], F32, kind="ExternalInput")
    out_rows = nc.dram_tensor("out_rows", [OPC, H], F32, kind="ExternalOutput")
    group_sizes = nc.dram_tensor("group_sizes", [E, 1], I32, kind="ExternalOutput")
    lb_loss = nc.dram_tensor("lb_loss", [1, 1], F32, kind="ExternalOutput")
    meta = nc.dram_tensor("meta", [NK, 2], F32)       # (src_token, weight) per out row

    with tile.TileContext(nc) as tc:
        with tc.tile_pool(name="persist", bufs=1) as pp:
            srcsel = pp.tile([P, OT], I32)
            wsel = pp.tile([P, OT], F32)
            cnt_i = pp.tile([E, 1], I32)
            lbl_sb = pp.tile([1, 1], F32)

            with (
                tc.tile_pool(name="route", bufs=1) as rp,
                tc.tile_pool(name="route_ps", bufs=3, space="PSUM") as rps,
                tc.tile_pool(name="route_ps1", bufs=1, space="PSUM") as rps1,
            ):
                # ---------- constants ----------
                id128 = rp.tile([P, P], F32)
                make_identity(nc, id128[:])
                id64 = rp.tile([E, E], F32)
                make_identity(nc, id64[:])
                ut64 = rp.tile([E, E], F32)
                make_upper_triangular(nc, ut64[:], val=1.0, diag=False)
                ones128 = rp.tile([P, 1], F32)
                nc.vector.memset(ones128[:], 1.0)
                zero64 = rp.tile([E, 1], F32)
                nc.vector.memset(zero64[:], 0.0)
                e64i = rp.tile([E, 1], I32)
                nc.gpsimd.iota(e64i[:], pattern=[[0, 1]], channel_multiplier=1)
                e64f = rp.tile([E, 1], F32)
                nc.vector.tensor_copy(e64f[:], e64i[:])
                r0sb = pp.tile([P, 1], F32)
                nc.sync.dma_start(r0sb[:], r0_d[:])

                # token id per (p, n): src = 128*n + p
                srci = rp.tile([P, NT], I32)
                nc.gpsimd.iota(srci[:], pattern=[[P, NT]], channel_multiplier=1)
                srcf = rp.tile([P, NT], F32)
                nc.vector.tensor_copy(srcf[:], srci[:])

                # ---------- softmax (unnormalized exp; N(0,1) logits can't overflow) ----------
                lg = rp.tile([P, NT, E], F32)
                nc.sync.dma_start(lg[:], router_logits[:].rearrange("(n p) e -> p n e", p=P))
                exps = rp.tile([P, NT, E], F32)
                nc.scalar.activation(exps[:], lg[:], AF.Exp)
                sums = rp.tile([P, NT], F32)
                nc.vector.tensor_reduce(sums[:], exps[:], axis=AX.X, op=OP.add)
                inv = rp.tile([P, NT], F32)
                nc.vector.reciprocal(inv[:], sums[:])
                probs = exps  # in-place
                nc.vector.tensor_tensor(
                    probs[:], exps[:],
                    inv[:].rearrange("p (n o) -> p n o", o=1).broadcast_to([P, NT, E]),
                    op=OP.mult)

                # ---------- top-8 ----------
                top8 = rp.tile([P, NT, K], F32)
                for n in range(NT):
                    nc.vector.max(out=top8[:, n, :], in_=probs[:, n, :])
                sum8 = rp.tile([P, NT], F32)
                nc.vector.tensor_reduce(sum8[:], top8[:], axis=AX.X, op=OP.add)
                inv8 = rp.tile([P, NT], F32)
                nc.vector.reciprocal(inv8[:], sum8[:])
                mask = rp.tile([P, NT, E], F32)
                nc.vector.tensor_tensor(
                    mask[:], probs[:],
                    top8[:, :, 7:8].broadcast_to([P, NT, E]),
                    op=OP.is_ge)
                wfull = rp.tile([P, NT, E], F32)
                nc.vector.tensor_tensor(
                    wfull[:], probs[:],
                    inv8[:].rearrange("p (n o) -> p n o", o=1).broadcast_to([P, NT, E]),
                    op=OP.mult)

                # ---------- w packed into an expert-keyed sort key ----------
                # Within a token, dest is strictly increasing in expert id, so
                # the dest-desc order of the main key equals expert-desc order.
                # Key e + w/2 carries w to ~2^-18 abs (output abs err ~2e-5).
                e_freei = rp.tile([P, E], I32)
                nc.gpsimd.iota(e_freei[:], pattern=[[1, E]], channel_multiplier=0)
                e_freef = rp.tile([P, E], F32)
                nc.vector.tensor_copy(e_freef[:], e_freei[:])
                ebc = e_freef[:].rearrange("p (o e) -> p o e", o=1).broadcast_to([P, NT, E])
                khi = wfull  # in-place
                nc.vector.scalar_tensor_tensor(
                    khi[:], wfull[:], 0.5, ebc, op0=OP.mult, op1=OP.add)
                nc.vector.tensor_tensor(khi[:], khi[:], mask[:], op=OP.mult)
                nc.vector.scalar_tensor_tensor(
                    khi[:], khi[:], -1.0, mask[:], op0=OP.add, op1=OP.add)
                khi8 = rp.tile([P, NT, K], F32)
                for n in range(NT):
                    nc.vector.max(out=khi8[:, n, :], in_=khi[:, n, :])
                e8i2 = rp.tile([P, NT, K], I32)
                nc.vector.tensor_copy(e8i2[:], khi8[:])   # rounds to e (frac < 0.5)
                e8f2 = rp.tile([P, NT, K], F32)
                nc.vector.tensor_copy(e8f2[:], e8i2[:])
                w8d = rp.tile([P, NT, K], F32)
                nc.vector.tensor_tensor(w8d[:], khi8[:], e8f2[:], op=OP.subtract)
                nc.vector.tensor_scalar(w8d[:], w8d[:], 2.0, None, op0=OP.mult)

                # ---------- transpose mask -> [E, N] ----------
                maskT = rp.tile([E, N], F32)
                for n in range(NT):
                    tp = rps.tile([E, P], F32, tag="tp", space="PSUM")
                    nc.tensor.transpose(tp[:], mask[:, n, :], id128[:])
                    nc.vector.tensor_copy(maskT[:, n * P:(n + 1) * P], tp[:])

                # ---------- global positions ----------
                ct = rp.tile([E, N], F32)
                nc.vector.tensor_tensor_scan(
                    ct[:], maskT[:], zero64[:].broadcast_to([E, N]),
                    initial=0.0, op0=OP.add, op1=OP.add)
                cnt_f = rp.tile([E, 1], F32)
                nc.vector.tensor_copy(cnt_f[:], ct[:, N - 1:N])
                nc.vector.tensor_copy(cnt_i[:], cnt_f[:])
                nc.sync.dma_start(group_sizes[:], cnt_i[:])
                b_ps = rps1.tile([E, 1], F32, tag="b", space="PSUM")
                nc.tensor.matmul(b_ps[:], lhsT=ut64[:], rhs=cnt_f[:], start=True, stop=True)
                # Bp = B + e/64 - 1; key = (ct + Bp)*maskT + (maskT - 1)
                bp = rp.tile([E, 1], F32)
                nc.vector.tensor_scalar(bp[:], e64f[:], 1.0 / 64.0, -1.0, op0=OP.mult, op1=OP.add)
                nc.vector.tensor_tensor(bp[:], bp[:], b_ps[:], op=OP.add)
                keyt = ct  # in-place
                nc.vector.scalar_tensor_tensor(
                    keyt[:], ct[:], bp[:], maskT[:], op0=OP.add, op1=OP.mult)
                nc.vector.scalar_tensor_tensor(
                    keyt[:], keyt[:], -1.0, maskT[:], op0=OP.add, op1=OP.add)

                # ---------- compact to [P, NT, K] via transpose + max8 ----------
                key8 = rp.tile([P, NT, K], F32)
                for n in range(NT):
                    kp = rps.tile([P, E], F32, tag="tp", space="PSUM")
                    nc.tensor.transpose(kp[:], keyt[:, n * P:(n + 1) * P], id64[:])
                    ks = rp.tile([P, E], F32, tag="ks")
                    nc.vector.tensor_copy(ks[:], kp[:])
                    nc.vector.max(out=key8[:, n, :], in_=ks[:])

                # key = dest + e/64 (desc by dest within a token).
                # 64*key = 64*dest + e is an exact integer < 2^23: cast, then
                # shift/mask (f32->i32 cast is round-to-nearest on HW, exact here).
                k64f = rp.tile([P, NT, K], F32)
                nc.vector.tensor_scalar(k64f[:], key8[:], 64.0, None, op0=OP.mult)
                k64i = rp.tile([P, NT, K], I32)
                nc.vector.tensor_copy(k64i[:], k64f[:])
                dest8i = rp.tile([P, NT, K], I32)
                nc.vector.tensor_scalar(dest8i[:], k64i[:], 6, None,
                                        op0=OP.logical_shift_right)

                # payload (src, e) per pair
                pay = rp.tile([P, NT * K, 2], F32)
                nc.vector.tensor_copy(
                    pay[:, :, 0].rearrange("p (n k) -> p n k", k=K),
                    srcf[:].rearrange("p (n o) -> p n o", o=1).broadcast_to([P, NT, K]))
                nc.vector.tensor_copy(
                    pay[:, :, 1], w8d[:].rearrange("p n k -> p (n k)"))

                # HW indirect DMA supports one offset per partition: one call
                # per (tile, k) pair slot. Issue all 512 back-to-back inside a
                # critical section — Tile's conservative whole-tensor WAW
                # tracking would otherwise serialize each call on the previous
                # one's HBM completion receipt. The exit drain waits for the
                # queue before the readback below runs.
                d8i2 = dest8i[:].rearrange("p n k -> p (n k)")
                scat_sem = nc.alloc_semaphore("scat_sem")
                with tc.tile_critical():
                    for s in range(NT * K):
                        nc.gpsimd.indirect_dma_start(
                            out=meta[:],
                            out_offset=bass.IndirectOffsetOnAxis(ap=d8i2[:, s:s + 1], axis=0),
                            in_=pay[:, s, :],
                            in_offset=None).then_inc(scat_sem, 16)
                    nc.gpsimd.wait_ge(scat_sem, NT * K * 16)

                # ---------- read back this core's range ----------
                offs_f = rp.tile([P, OT], F32)
                nc.vector.tensor_scalar(offs_f[:], srcf[:, :OT], r0sb[:], None, op0=OP.add)
                offs = rp.tile([P, OT], I32)
                nc.vector.tensor_copy(offs[:], offs_f[:])
                metag = rp.tile([P, OT, 2], F32)
                for j in range(OT):
                    nc.gpsimd.indirect_dma_start(
                        out=metag[:, j, :],
                        out_offset=None,
                        in_=meta[:],
                        in_offset=bass.IndirectOffsetOnAxis(ap=offs[:, j:j + 1], axis=0))
                nc.vector.tensor_copy(wsel[:], metag[:, :, 1])
                nc.vector.tensor_copy(srcsel[:], metag[:, :, 0])

                # ---------- lb loss ----------
                pe_ps = rps1.tile([E, 1], F32, tag="pe", space="PSUM")
                for n in range(NT):
                    nc.tensor.matmul(pe_ps[:], lhsT=probs[:, n, :], rhs=ones128[:],
                                     start=(n == 0), stop=(n == NT - 1))
                prod = rp.tile([E, 1], F32)
                nc.vector.tensor_tensor(prod[:], pe_ps[:], cnt_f[:], op=OP.mult)
                pr_ps = rps1.tile([1, E], F32, tag="pr", space="PSUM")
                nc.tensor.transpose(pr_ps[:], prod[:], id64[:])
                prt = rp.tile([1, E], F32)
                nc.vector.tensor_copy(prt[:], pr_ps[:])
                red = rp.tile([1, 1], F32)
                nc.vector.tensor_reduce(red[:], prt[:], axis=AX.X, op=OP.add)
                coef = LBL_COEF * (E / K) / (float(NK) * float(N))
                nc.vector.tensor_scalar(lbl_sb[:], red[:], coef, None, op0=OP.mult)
                nc.sync.dma_start(lb_loss[:], lbl_sb[:])

            # ---------- main gather/scale/store ----------
            with tc.tile_pool(name="main", bufs=4) as mp:
                for j in range(OT):
                    rows = mp.tile([P, H], F32, tag="rows")
                    nc.gpsimd.indirect_dma_start(
                        out=rows[:],
                        out_offset=None,
                        in_=inputs_flat[:],
                        in_offset=bass.IndirectOffsetOnAxis(ap=srcsel[:, j:j + 1], axis=0))
                    scaled = mp.tile([P, H], F32, tag="scaled")
                    if j % scale_split == 0:
                        nc.scalar.activation(scaled[:], rows[:], AF.Copy,
                                             scale=wsel[:, j:j + 1])
                    else:
                        nc.vector.tensor_scalar_mul(scaled[:], rows[:], wsel[:, j:j + 1])
                    nc.sync.dma_start(out_rows[j * P:(j + 1) * P, :], scaled[:])
    return nc


_NC_CACHE = {}


def _get_nc():
    if "nc" not in _NC_CACHE:
        nc = bacc.Bacc("TRN2", target_bir_lowering=False)
        _build(nc)
        nc.compile()
        _NC_CACHE["nc"] = nc
    return _NC_CACHE["nc"]


def kernel(inputs_flat, router_logits, _trace=False, **_ignored):
    inputs_flat = np.ascontiguousarray(np.asarray(inputs_flat, dtype=np.float32))
    router_logits = np.ascontiguousarray(np.asarray(router_logits, dtype=np.float32))
    assert inputs_flat.shape == (N, H) and router_logits.shape == (N, E)

    nc = _get_nc()
    in_maps = [
        {
            "inputs_flat": inputs_flat,
            "router_logits": router_logits,
            "r0": np.full((P, 1), c * OPC, np.float32),
        }
        for c in range(C)
    ]
    res = run_bass_kernel_spmd(nc, in_maps, core_ids=list(range(C)), trace=_trace)
    out = np.concatenate([res.results[c]["out_rows"] for c in range(C)], axis=0)
    gs = res.results[0]["group_sizes"].reshape(E).astype(np.int32)
    lbl = np.float32(res.results[0]["lb_loss"].reshape(()))
    if _trace:
        kernel.last_result = res
    return out, gs, lbl


# revision 15
# speedup vs baseline: 1.1671x; 1.1671x over previous
"""MoE routing kernel (softmax -> top-8 -> sort/dispatch by expert) on 8 TRN2 cores.

Strategy: every core computes the full routing (cheap: [8192, 64]) and builds the
global expert-sorted metadata on-device; output rows are sharded by global output
position (8192 rows per core), so each core does an indirect row gather + scale +
contiguous store for its slice and the host just concatenates.
"""

import numpy as np

import concourse.bacc as bacc
import concourse.bass as bass
import concourse.mybir as mybir
import concourse.tile as tile
from concourse.bass_utils import run_bass_kernel_spmd
from concourse.masks import make_identity, make_upper_triangular

F32 = mybir.dt.float32
I32 = mybir.dt.int32
AF = mybir.ActivationFunctionType
OP = mybir.AluOpType
AX = mybir.AxisListType

N = 8192          # tokens
E = 64            # experts
K = 8             # top-k
H = 2048          # hidden
P = 128
NT = N // P       # token tiles
C = 8             # cores
NK = N * K        # total dispatched rows
OPC = NK // C     # out rows per core
OT = OPC // P     # out tiles per core
LBL_COEF = 0.01


def _build(nc, scale_split=2):
    inputs_flat = nc.dram_tensor("inputs_flat", [N, H], F32, kind="ExternalInput")
    router_logits = nc.dram_tensor("router_logits", [N, E], F32, kind="ExternalInput")
    r0_d = nc.dram_tensor("r0", [P, 1], F32, kind="ExternalInput")
    out_rows = nc.dram_tensor("out_rows", [OPC, H], F32, kind="ExternalOutput")
    group_sizes = nc.dram_tensor("group_sizes", [E, 1], I32, kind="ExternalOutput")
    lb_loss = nc.dram_tensor("lb_loss", [1, 1], F32, kind="ExternalOutput")
    meta = nc.dram_tensor("meta", [NK, 2], F32)       # (src_token, weight) per out row

    with tile.TileContext(nc) as tc:
        with tc.tile_pool(name="persist", bufs=1) as pp:
            srcsel = pp.tile([P, OT], I32)
            metag = pp.tile([P, OT, 2], F32)
            cnt_i = pp.tile([E, 1], I32)
            lbl_sb = pp.tile([1, 1], F32)

            with (
                tc.tile_pool(name="route", bufs=1) as rp,
                tc.tile_pool(name="route_ps", bufs=3, space="PSUM") as rps,
                tc.tile_pool(name="route_ps1", bufs=1, space="PSUM") as rps1,
            ):
                # ---------- constants ----------
                id128 = rp.tile([P, P], F32)
                make_identity(nc, id128[:])
                id64 = rp.tile([E, E], F32)
                make_identity(nc, id64[:])
                ut64 = rp.tile([E, E], F32)
                make_upper_triangular(nc, ut64[:], val=1.0, diag=False)
                ones128 = rp.tile([P, 1], F32)
                nc.vector.memset(ones128[:], 1.0)
                zero64 = rp.tile([E, 1], F32)
                nc.vector.memset(zero64[:], 0.0)
                e64i = rp.tile([E, 1], I32)
                nc.gpsimd.iota(e64i[:], pattern=[[0, 1]], channel_multiplier=1)
                e64f = rp.tile([E, 1], F32)
                nc.vector.tensor_copy(e64f[:], e64i[:])
                r0sb = pp.tile([P, 1], F32)
                nc.sync.dma_start(r0sb[:], r0_d[:])

                # token id per (p, n): src = 128*n + p
                srci = rp.tile([P, NT], I32)
                nc.gpsimd.iota(srci[:], pattern=[[P, NT]], channel_multiplier=1)
                srcf = rp.tile([P, NT], F32)
                nc.vector.tensor_copy(srcf[:], srci[:])

                # ---------- softmax (unnormalized exp; N(0,1) logits can't overflow) ----------
                lg = rp.tile([P, NT, E], F32)
                nc.sync.dma_start(lg[:], router_logits[:].rearrange("(n p) e -> p n e", p=P))
                exps = rp.tile([P, NT, E], F32)
                nc.scalar.activation(exps[:], lg[:], AF.Exp)
                sums = rp.tile([P, NT], F32)
                nc.vector.tensor_reduce(sums[:], exps[:], axis=AX.X, op=OP.add)
                inv = rp.tile([P, NT], F32)
                nc.vector.reciprocal(inv[:], sums[:])
                probs = exps  # in-place
                nc.vector.tensor_tensor(
                    probs[:], exps[:],
                    inv[:].rearrange("p (n o) -> p n o", o=1).broadcast_to([P, NT, E]),
                    op=OP.mult)

                # ---------- top-8 ----------
                top8 = rp.tile([P, NT, K], F32)
                for n in range(NT):
                    nc.vector.max(out=top8[:, n, :], in_=probs[:, n, :])
                sum8 = rp.tile([P, NT], F32)
                nc.vector.tensor_reduce(sum8[:], top8[:], axis=AX.X, op=OP.add)
                inv8 = rp.tile([P, NT], F32)
                nc.vector.reciprocal(inv8[:], sum8[:])
                mask = rp.tile([P, NT, E], F32)
                nc.vector.tensor_tensor(
                    mask[:], probs[:],
                    top8[:, :, 7:8].broadcast_to([P, NT, E]),
                    op=OP.is_ge)
                wfull = rp.tile([P, NT, E], F32)
                nc.vector.tensor_tensor(
                    wfull[:], probs[:],
                    inv8[:].rearrange("p (n o) -> p n o", o=1).broadcast_to([P, NT, E]),
                    op=OP.mult)

                # ---------- w packed into an expert-keyed sort key ----------
                # Within a token, dest is strictly increasing in expert id, so
                # the dest-desc order of the main key equals expert-desc order.
                # Key e + w/2 carries w to ~2^-18 abs (output abs err ~2e-5).
                e_freei = rp.tile([P, E], I32)
                nc.gpsimd.iota(e_freei[:], pattern=[[1, E]], channel_multiplier=0)
                e_freef = rp.tile([P, E], F32)
                nc.vector.tensor_copy(e_freef[:], e_freei[:])
                ebc = e_freef[:].rearrange("p (o e) -> p o e", o=1).broadcast_to([P, NT, E])
                khi = wfull  # in-place
                nc.vector.scalar_tensor_tensor(
                    khi[:], wfull[:], 0.5, ebc, op0=OP.mult, op1=OP.add)
                nc.vector.tensor_tensor(khi[:], khi[:], mask[:], op=OP.mult)
                nc.vector.scalar_tensor_tensor(
                    khi[:], khi[:], -1.0, mask[:], op0=OP.add, op1=OP.add)
                khi8 = rp.tile([P, NT, K], F32)
                for n in range(NT):
                    nc.vector.max(out=khi8[:, n, :], in_=khi[:, n, :])
                e8i2 = rp.tile([P, NT, K], I32)
                nc.vector.tensor_copy(e8i2[:], khi8[:])   # rounds to e (frac < 0.5)
                e8f2 = rp.tile([P, NT, K], F32)
                nc.vector.tensor_copy(e8f2[:], e8i2[:])
                w8d = rp.tile([P, NT, K], F32)
                nc.vector.tensor_tensor(w8d[:], khi8[:], e8f2[:], op=OP.subtract)
                nc.vector.tensor_scalar(w8d[:], w8d[:], 2.0, None, op0=OP.mult)

                # ---------- transpose mask -> [E, N] ----------
                maskT = rp.tile([E, N], F32)
                for n in range(NT):
                    tp = rps.tile([E, P], F32, tag="tp", space="PSUM")
                    nc.tensor.transpose(tp[:], mask[:, n, :], id128[:])
                    nc.scalar.activation(maskT[:, n * P:(n + 1) * P], tp[:], AF.Copy)

                # ---------- global positions ----------
                ct = rp.tile([E, N], F32)
                nc.vector.tensor_tensor_scan(
                    ct[:], maskT[:], zero64[:].broadcast_to([E, N]),
                    initial=0.0, op0=OP.add, op1=OP.add)
                cnt_f = rp.tile([E, 1], F32)
                nc.vector.tensor_copy(cnt_f[:], ct[:, N - 1:N])
                nc.vector.tensor_copy(cnt_i[:], cnt_f[:])
                nc.sync.dma_start(group_sizes[:], cnt_i[:])
                b_ps = rps1.tile([E, 1], F32, tag="b", space="PSUM")
                nc.tensor.matmul(b_ps[:], lhsT=ut64[:], rhs=cnt_f[:], start=True, stop=True)
                # Bp = B + e/64 - 1; key = (ct + Bp)*maskT + (maskT - 1)
                bp = rp.tile([E, 1], F32)
                nc.vector.tensor_scalar(bp[:], e64f[:], 1.0 / 64.0, -1.0, op0=OP.mult, op1=OP.add)
                nc.vector.tensor_tensor(bp[:], bp[:], b_ps[:], op=OP.add)
                keyt = ct  # in-place
                nc.vector.scalar_tensor_tensor(
                    keyt[:], ct[:], bp[:], maskT[:], op0=OP.add, op1=OP.mult)
                nc.vector.scalar_tensor_tensor(
                    keyt[:], keyt[:], -1.0, maskT[:], op0=OP.add, op1=OP.add)

                # ---------- compact to [P, NT, K] via transpose + max8 ----------
                key8 = rp.tile([P, NT, K], F32)
                for n in range(NT):
                    kp = rps.tile([P, E], F32, tag="tp", space="PSUM")
                    nc.tensor.transpose(kp[:], keyt[:, n * P:(n + 1) * P], id64[:])
                    ks = rp.tile([P, E], F32, tag="ks")
                    nc.scalar.activation(ks[:], kp[:], AF.Copy)
                    nc.vector.max(out=key8[:, n, :], in_=ks[:])

                # key = dest + e/64 (desc by dest within a token).
                # 64*key = 64*dest + e is an exact integer < 2^23: cast, then
                # shift/mask (f32->i32 cast is round-to-nearest on HW, exact here).
                k64f = rp.tile([P, NT, K], F32)
                nc.vector.tensor_scalar(k64f[:], key8[:], 64.0, None, op0=OP.mult)
                k64i = rp.tile([P, NT, K], I32)
                nc.vector.tensor_copy(k64i[:], k64f[:])
                dest8i = rp.tile([P, NT, K], I32)
                nc.vector.tensor_scalar(dest8i[:], k64i[:], 6, None,
                                        op0=OP.logical_shift_right)

                # payload (src, e) per pair
                pay = rp.tile([P, NT * K, 2], F32)
                nc.vector.tensor_copy(
                    pay[:, :, 0].rearrange("p (n k) -> p n k", k=K),
                    srcf[:].rearrange("p (n o) -> p n o", o=1).broadcast_to([P, NT, K]))
                nc.vector.tensor_copy(
                    pay[:, :, 1], w8d[:].rearrange("p n k -> p (n k)"))

                # HW indirect DMA supports one offset per partition: one call
                # per (tile, k) pair slot. Issue all 512 back-to-back inside a
                # critical section — Tile's conservative whole-tensor WAW
                # tracking would otherwise serialize each call on the previous
                # one's HBM completion receipt. The exit drain waits for the
                # queue before the readback below runs.
                d8i2 = dest8i[:].rearrange("p n k -> p (n k)")
                scat_sem = nc.alloc_semaphore("scat_sem")
                with tc.tile_critical():
                    for s in range(NT * K):
                        nc.gpsimd.indirect_dma_start(
                            out=meta[:],
                            out_offset=bass.IndirectOffsetOnAxis(ap=d8i2[:, s:s + 1], axis=0),
                            in_=pay[:, s, :],
                            in_offset=None).then_inc(scat_sem, 16)
                    nc.gpsimd.wait_ge(scat_sem, NT * K * 16)

                # ---------- read back this core's range ----------
                offs_f = rp.tile([P, OT], F32)
                nc.vector.tensor_scalar(offs_f[:], srcf[:, :OT], r0sb[:], None, op0=OP.add)
                offs = rp.tile([P, OT], I32)
                nc.vector.tensor_copy(offs[:], offs_f[:])
                for j in range(OT):
                    nc.gpsimd.indirect_dma_start(
                        out=metag[:, j, :],
                        out_offset=None,
                        in_=meta[:],
                        in_offset=bass.IndirectOffsetOnAxis(ap=offs[:, j:j + 1], axis=0))


                # ---------- lb loss ----------
                pe_ps = rps1.tile([E, 1], F32, tag="pe", space="PSUM")
                for n in range(NT):
                    nc.tensor.matmul(pe_ps[:], lhsT=probs[:, n, :], rhs=ones128[:],
                                     start=(n == 0), stop=(n == NT - 1))
                prod = rp.tile([E, 1], F32)
                nc.vector.tensor_tensor(prod[:], pe_ps[:], cnt_f[:], op=OP.mult)
                pr_ps = rps1.tile([1, E], F32, tag="pr", space="PSUM")
                nc.tensor.transpose(pr_ps[:], prod[:], id64[:])
                prt = rp.tile([1, E], F32)
                nc.vector.tensor_copy(prt[:], pr_ps[:])
                red = rp.tile([1, 1], F32)
                nc.vector.tensor_reduce(red[:], prt[:], axis=AX.X, op=OP.add)
                coef = LBL_COEF * (E / K) / (float(NK) * float(N))
                nc.vector.tensor_scalar(lbl_sb[:], red[:], coef, None, op0=OP.mult)
                nc.sync.dma_start(lb_loss[:], lbl_sb[:])

            # ---------- main gather/scale/store ----------
            with tc.tile_pool(name="main", bufs=4) as mp:
                for j in range(OT):
                    nc.vector.tensor_copy(srcsel[:, j:j + 1], metag[:, j, 0:1])
                    rows = mp.tile([P, H], F32, tag="rows")
                    nc.gpsimd.indirect_dma_start(
                        out=rows[:],
                        out_offset=None,
                        in_=inputs_flat[:],
                        in_offset=bass.IndirectOffsetOnAxis(ap=srcsel[:, j:j + 1], axis=0))
                    scaled = mp.tile([P, H], F32, tag="scaled")
                    wj = metag[:, j, 1:2]
                    if j % scale_split == 0:
                        nc.scalar.activation(scaled[:], rows[:], AF.Copy, scale=wj)
                    else:
                        nc.vector.tensor_scalar_mul(scaled[:], rows[:], wj)
                    nc.sync.dma_start(out_rows[j * P:(j + 1) * P, :], scaled[:])
    return nc


_NC_CACHE = {}


def _get_nc():
    if "nc" not in _NC_CACHE:
        nc = bacc.Bacc("TRN2", target_bir_lowering=False)
        _build(nc)
        nc.compile()
        _NC_CACHE["nc"] = nc
    return _NC_CACHE["nc"]


def kernel(inputs_flat, router_logits, _trace=False, **_ignored):
    inputs_flat = np.ascontiguousarray(np.asarray(inputs_flat, dtype=np.float32))
    router_logits = np.ascontiguousarray(np.asarray(router_logits, dtype=np.float32))
    assert inputs_flat.shape == (N, H) and router_logits.shape == (N, E)

    nc = _get_nc()
    in_maps = [
        {
            "inputs_flat": inputs_flat,
            "router_logits": router_logits,
            "r0": np.full((P, 1), c * OPC, np.float32),
        }
        for c in range(C)
    ]
    res = run_bass_kernel_spmd(nc, in_maps, core_ids=list(range(C)), trace=_trace)
    out = np.concatenate([res.results[c]["out_rows"] for c in range(C)], axis=0)
    gs = res.results[0]["group_sizes"].reshape(E).astype(np.int32)
    lbl = np.float32(res.results[0]["lb_loss"].reshape(()))
    if _trace:
        kernel.last_result = res
    return out, gs, lbl
